# revision 1
# baseline (speedup 1.0000x reference)
"""Correlation cost-volume kernel for Trainium2 (8 NeuronCores).

out[b, dy*9+dx, y, x] = mean_c input1[b,c,y,x] * pad(input2)[b,c,y+dy,x+dx]

Sharding: pure data parallel over batch (B=8 -> 1 batch element per core).

Per core, per output row y and x-tile of 128:
  - 3 fp32r matmuls (lhsT = in1 columns, rhs = 3 dy-rows x 136 halo cols of
    padded in2) -> PSUM Gram tiles G[m, dy, i] = sum_c in1[c,x0+m] in2p[c,y+dy,x0+i]
  - one scaled copy (x 1/C) PSUM -> SBUF
  - two windowed DMAs dump the 72-wide band neighborhoods (per 64-row group)
    to DRAM; the host extracts the 9-diagonal band out[.., x0+m] = G[m, dy, m+dx]
    (a pure indexing step, done in numpy during unshard).

The toolchain here rejects instructions with >1 sync wait, so after tracing we
split extra waits onto same-engine NoOps (split_multi_waits).
"""
import numpy as np

B, C, H, W = 8, 128, 128, 256
PAD = 4
ND = 9            # displacements per axis
YB = 32           # y-block rows per load
NYB = H // YB
NH = W + 2 * PAD  # padded in2 row length: 264
GW = 136          # gram halo width per x-tile: 128 + 8
SL = ND * GW      # S_all row length: 1224

_CACHE = {}


def _build():
    import concourse.bass as bass
    import concourse.mybir as mybir
    import bass_rust
    from concourse.ap import AP
    from concourse.tile import TileContext

    f32 = mybir.dt.float32
    f32r = mybir.dt.float32r

    nc = bass.Bass()
    IN1 = nc.dram_tensor("in1", [C, H, W], f32, kind="ExternalInput")
    IN2P = nc.dram_tensor("in2p", [C, H + 8, NH], f32, kind="ExternalInput")
    # dump: [y, xt, group, 64 rows, 9 dy, 72 cols]
    OUTD = nc.dram_tensor("outd", [H, 2, 2, 64, ND, 72], f32,
                          kind="ExternalOutput")

    with TileContext(nc) as tc:
        with tc.tile_pool(name="pin1", bufs=2) as pin1, \
             tc.tile_pool(name="pin2", bufs=2) as pin2, \
             tc.tile_pool(name="psum", bufs=2, space="PSUM") as psum, \
             tc.tile_pool(name="ps", bufs=3) as ps_s:
            for yb in range(NYB):
                y0 = yb * YB
                t1 = pin1.tile([C, YB * W], f32r)
                t2 = pin2.tile([C, (YB + 8) * NH], f32r)
                nc.gpsimd.dma_start(
                    out=t1[:], in_=IN1[:, y0:y0 + YB, :].rearrange(
                        "c a b -> c (a b)"))
                nc.gpsimd.dma_start(
                    out=t2[:], in_=IN2P[:, y0:y0 + YB + 8, :].rearrange(
                        "c a b -> c (a b)"))
                t1t, t1o = t1[:].tensor, t1[:].offset
                t2t, t2o = t2[:].tensor, t2[:].offset
                for yy in range(YB):
                    y = y0 + yy
                    for xt in range(2):
                        x0 = xt * 128
                        pst_ = psum.tile([128, 1536], f32)
                        sall = ps_s.tile([128, SL], f32)
                        lhsT = AP(t1t, t1o + yy * W + x0, [[YB * W, C], [1, 128]])
                        for mmi in range(3):
                            rhs = AP(t2t, t2o + (yy + mmi * 3) * NH + x0,
                                     [[(YB + 8) * NH, C], [NH, 3], [1, GW]])
                            nc.tensor.matmul(
                                pst_[:, mmi * 512: mmi * 512 + 3 * GW],
                                lhsT, rhs, start=True, stop=True)
                        pt, po = pst_[:].tensor, pst_[:].offset
                        st, so = sall[:].tensor, sall[:].offset
                        src = AP(pt, po, [[1536, 128], [512, 3], [GW, 3], [1, GW]])
                        dst = AP(st, so, [[SL, 128], [3 * GW, 3], [GW, 3], [1, GW]])
                        if xt == 0:
                            nc.scalar.mul(dst, src, 1.0 / C)
                        else:
                            nc.vector.tensor_scalar_mul(dst, src, 1.0 / C)
                        for g in range(2):
                            soff = so + g * 64 * SL + g * 64
                            dsrc = AP(st, soff, [[SL, 64], [GW, ND], [1, 72]])
                            ddst = OUTD[y, xt, g].rearrange("a b c -> a b c")
                            nc.sync.dma_start(out=ddst, in_=dsrc)

    # --- split multi-wait instructions (this walrus accepts max 1) ---
    n = 0
    for fn in nc.m.functions:
        for blk in fn.blocks:
            il = blk.instructions
            new = []
            changed = False
            for ins in il:
                si = ins.sync_info
                if si is not None and len(si.on_wait) > 1:
                    waits = list(si.on_wait)
                    for w in waits[:-1]:
                        n += 1
                        new.append(bass_rust.InstNoOp(
                            name=f"wsplit_{n}", engine=ins.engine,
                            sync_info=bass_rust.SyncInfo(
                                on_wait=[w], on_update=[])))
                    si.on_wait = waits[-1:]
                    ins.sync_info = si
                    changed = True
                new.append(ins)
            if changed:
                blk.instructions = new
    return nc


def _get_nc():
    if "nc" not in _CACHE:
        _CACHE["nc"] = _build()
    return _CACHE["nc"]


# band gather index: j[m', dy, dx] = m' + dx  (into the 72-wide window)
_JIDX = (np.arange(64)[:, None, None] + np.arange(ND)[None, None, :])  # [64,1->9,9]
_JIDX = np.broadcast_to(_JIDX, (64, ND, ND))


def kernel(input1: np.ndarray, input2: np.ndarray) -> np.ndarray:
    from concourse.bass_utils import run_bass_kernel_spmd

    input1 = np.ascontiguousarray(input1, dtype=np.float32)
    input2 = np.ascontiguousarray(input2, dtype=np.float32)
    in_maps = []
    for b in range(B):
        in2p = np.pad(input2[b], ((0, 0), (PAD, PAD), (PAD, PAD)))
        in_maps.append({"in1": np.ascontiguousarray(input1[b]),
                        "in2p": np.ascontiguousarray(in2p)})

    nc = _get_nc()
    results = run_bass_kernel_spmd(nc, in_maps, core_ids=list(range(B))).results

    out = np.empty((B, ND * ND, H, W), dtype=np.float32)
    mi = np.arange(64)[:, None, None]
    dyi = np.arange(ND)[None, :, None]
    for b in range(B):
        D = results[b]["outd"]  # [H, 2, 2, 64, 9, 72]
        # band[y, xt, g, m', dy, dx] = D[y, xt, g, m', dy, m'+dx]
        band = D[:, :, :, mi, dyi, _JIDX]       # [H, 2, 2, 64, 9, 9]
        # out[d, y, x]: d = dy*9+dx, x = xt*128 + g*64 + m'
        band = band.transpose(4, 5, 0, 1, 2, 3)  # [9dy, 9dx, H, 2xt, 2g, 64]
        out[b] = band.reshape(ND * ND, H, W)
    return out



# revision 9
# speedup vs baseline: 1.9683x; 1.9683x over previous
"""Correlation cost-volume kernel for Trainium2 (8 NeuronCores).

out[b, d, y, x] = mean_c in1[b,c,y,x] * pad(in2)[b,c,y+dy,x+dx],
d = (dy+4)*9 + (dx+4), 81 displacements.

Sharding: pure data parallel over batch (B=8 -> 1 batch element per core).

Per core: both inputs live in SBUF whole, as bf16, column-major ([c, x, y]);
in1 is pre-scaled by 1/C on the host (exact exponent shift in bf16).

For each output column x (256 of them):
  - 4 col-tiled matmuls (tile_position=(0,32t)): stationary = in1T[:, x, 32t:32t+32]
    (128c x 32y), moving = in2T[:, x:x+9, 32t:32t+40] as 360 columns ordered
    (y' outer, dx inner) -> PSUM P[y, j]. For partition y = 32t+u the 81 band
    values land CONTIGUOUSLY at j in [9u, 9u+81), already in d-order.
  - Groups of GX=4 x share one 4-bank PSUM tile; one DVE/ACT copy (alternating
    engines) evacuates PSUM -> bf16 SBUF stage (PSUM reads are 1x; a single big
    copy amortizes the per-instruction bubble).
  - Stages hold GS=16 x; 32 per-u band DMAs per stage (pure-partition strides)
    write the compact [4t, 16x, 81d] slices: 5.3 MB output traffic vs 85 MB.

Host side: transpose/cast inputs, final [x,y,d] -> [d,y,x] permute + f32 cast.

The toolchain rejects instructions with >1 sync wait, so after tracing we
split extra waits onto same-engine NoOps (split_multi_waits).
"""
import numpy as np

B, C, H, W = 8, 128, 128, 256
PAD = 4
ND = 9             # displacements per axis
NDISP = ND * ND    # 81
HP = H + 2 * PAD   # 136 padded column height
NH = W + 2 * PAD   # 264 padded row width
SW = 40 * ND       # 360 psum stream width per x-column
GX = 4             # x-columns per psum tile (4 banks)
GS = 16            # x-columns per stage tile

_CACHE = {}


def _build(split_waits: bool = True, sim_mode: bool = False):
    import concourse.bass as bass
    import concourse.mybir as mybir
    import bass_rust
    from concourse.ap import AP
    from concourse.tile import TileContext

    f32 = mybir.dt.float32
    bf16 = mybir.dt.bfloat16

    nc = bass.Bass()
    # in1t[c, x*H + y] = in1[c, y, x] / C     (bf16)
    IN1T = nc.dram_tensor("in1t", [C, W * H], bf16, kind="ExternalInput")
    # in2t[c, xi*HP + yi] = pad(in2)[c, yi, xi]  (bf16)
    IN2T = nc.dram_tensor("in2t", [C, NH * HP], bf16, kind="ExternalInput")
    # outd[x, t, u, d] = out[d, 32t+u, x]   (bf16)
    OUT = nc.dram_tensor("outd", [W, 4, 32, NDISP], bf16, kind="ExternalOutput")
    OP_T = 32 * NDISP          # 2592
    OP_X = 4 * OP_T            # 10368
    if sim_mode:
        # sim's AP checker can't view the per-u band APs; dump raw stages
        OUTF = nc.dram_tensor("outf", [W // GS, 128, GS * SW], bf16,
                              kind="ExternalOutput")

    with TileContext(nc) as tc:
        with tc.tile_pool(name="pin", bufs=1) as pin, \
             tc.tile_pool(name="ppsum", bufs=2, space="PSUM") as ppsum, \
             tc.tile_pool(name="pstage", bufs=2) as pstage:
            t1 = pin.tile([C, W * H], bf16)
            t2 = pin.tile([C, NH * HP], bf16)
            NCH = 8
            for i in range(NCH):
                c0 = i * (W // NCH) * H
                c1 = (i + 1) * (W // NCH) * H
                nc.gpsimd.dma_start(out=t1[:, c0:c1], in_=IN1T[:, c0:c1])
            for i in range(NCH):
                c0 = i * (NH // NCH) * HP
                c1 = (i + 1) * (NH // NCH) * HP
                nc.gpsimd.dma_start(out=t2[:, c0:c1], in_=IN2T[:, c0:c1])
            t1t, t1o = t1[:].tensor, t1[:].offset
            t2t, t2o = t2[:].tensor, t2[:].offset
            P1 = t1[:].ap[0][0]
            P2 = t2[:].ap[0][0]

            for s in range(W // GS):
                S = pstage.tile([128, GS * SW], bf16)
                st, so = S[:].tensor, S[:].offset
                SP = S[:].ap[0][0]
                for gg in range(GS // GX):
                    g = s * (GS // GX) + gg
                    x0 = GX * g
                    P = ppsum.tile([128, 2048], f32)
                    PP = P[:].ap[0][0]
                    for xi in range(GX):
                        x = x0 + xi
                        for t in range(4):
                            lhsT = AP(t1t, t1o + x * H + 32 * t,
                                      [[P1, C], [1, 32]])
                            rhs = AP(t2t, t2o + x * HP + 32 * t,
                                     [[P2, C], [1, 40], [HP, ND]])
                            nc.tensor.matmul(
                                P[32 * t:32 * t + 32, 512 * xi:512 * xi + SW],
                                lhsT, rhs, start=True, stop=True,
                                tile_position=(0, 32 * t))
                    pt, po = P[:].tensor, P[:].offset
                    csrc = AP(pt, po, [[PP, 128], [512, GX], [1, SW]])
                    cdst = AP(st, so + gg * GX * SW,
                              [[SP, 128], [SW, GX], [1, SW]])
                    if g % 2 == 0:
                        nc.scalar.copy(cdst, csrc)
                    else:
                        nc.vector.tensor_scalar_mul(cdst, csrc, 1.0)
                # 32 per-u band dumps (partition strides pure, cols in-bounds)
                xs0 = s * GS
                if sim_mode:
                    nc.sync.dma_start(out=OUTF[s], in_=S[:])
                else:
                    for u in range(32):
                        dsrc = AP(st, so + u * SP + 9 * u,
                                  [[32 * SP, 4], [SW, GS], [1, NDISP]])
                        ddst = AP(OUT[:].tensor, xs0 * OP_X + u * NDISP,
                                  [[OP_T, 4], [OP_X, GS], [1, NDISP]])
                        nc.sync.dma_start(out=ddst, in_=dsrc)

    # --- split multi-wait instructions (this walrus accepts max 1) ---
    if not split_waits:
        return nc
    n = 0
    for fn in nc.m.functions:
        for blk in fn.blocks:
            il = blk.instructions
            new = []
            changed = False
            for ins in il:
                si = ins.sync_info
                if si is not None and len(si.on_wait) > 1:
                    waits = list(si.on_wait)
                    for w in waits[:-1]:
                        n += 1
                        new.append(bass_rust.InstNoOp(
                            name=f"wsplit_{n}", engine=ins.engine,
                            sync_info=bass_rust.SyncInfo(
                                on_wait=[w], on_update=[])))
                    si.on_wait = waits[-1:]
                    ins.sync_info = si
                    changed = True
                new.append(ins)
            if changed:
                blk.instructions = new
    return nc


def _get_nc():
    if "nc" not in _CACHE:
        _CACHE["nc"] = _build()
    return _CACHE["nc"]


def _prep_core(in1_b: np.ndarray, in2_b: np.ndarray) -> dict:
    import ml_dtypes
    bf = ml_dtypes.bfloat16
    in1t = (np.asarray(in1_b).transpose(0, 2, 1) * np.float32(1.0 / C)).astype(bf)
    in2p = np.pad(np.asarray(in2_b), ((0, 0), (PAD, PAD), (PAD, PAD)))
    in2t = in2p.transpose(0, 2, 1).astype(bf)
    return {"in1t": np.ascontiguousarray(in1t).reshape(C, W * H),
            "in2t": np.ascontiguousarray(in2t).reshape(C, NH * HP)}


def kernel(input1: np.ndarray, input2: np.ndarray) -> np.ndarray:
    from concourse.bass_utils import run_bass_kernel_spmd

    input1 = np.ascontiguousarray(input1, dtype=np.float32)
    input2 = np.ascontiguousarray(input2, dtype=np.float32)
    in_maps = [_prep_core(input1[b], input2[b]) for b in range(B)]

    nc = _get_nc()
    results = run_bass_kernel_spmd(nc, in_maps, core_ids=list(range(B))).results

    out = np.empty((B, NDISP, H, W), dtype=np.float32)
    for b in range(B):
        D = results[b]["outd"].astype(np.float32)    # [W, 4, 32, 81]
        out[b] = D.reshape(W, H, NDISP).transpose(2, 1, 0)
    return out
